# revision 16
# baseline (speedup 1.0000x reference)
"""Trainium2 Bass kernel for the quirky multi-head attention problem.

Math (per batch b, head a), faithful to the reference:
    K = x[b] @ W_K[a].T          # [S, H]
    Q = x[b] @ W_Q[a].T
    V = x[b] @ W_V[a].T
    s[c, C] = (K @ Q.T)[c, C] / sqrt(H)        rows c = "key" index
    valid iff C <= c (tril); softmax over C per row c
    E = exp(s) * tril            # no max-subtraction: |s| <= ~7, fp32-safe
    denom[c] = sum_C E[c, C]
    z[C, h] = sum_c E[c, C] * (V/denom)[c, h]  # = E.T @ (V/denom)
    out[b] += z @ W_O[a].T

Sharding: 8 cores = 2 batches x 4 head-pairs. Each core handles one batch
and two heads; the attention matrix is device-local. Host sums the four
head-pair partial outputs per batch.

Device layouts (per core):
    xT   [E=512, S]   x[b] transposed (host-side)
    wk/wq/wv [E, 128] W[a0].T | W[a1].T concat on head axis
    wo0/wo1 [128, E]  W_O[a].T duplicated on both partition halves
    maskb [128, 896]  additive causal mask bands (0 or -1e9)
    outT [E, S]       partial output, transposed

On-chip flow per head: scores [c_blk=128 rows, 512-wide C chunks] are
matmul'd into rotating PSUM wave tiles (2 banks x 2 bufs), the diagonal
chunk gets an additive -1e9 mask, ScalarE applies exp (scale=1/8) writing
the row panel to SBUF with a fused per-row accumulation (softmax denom).
z^T [64, S] accumulates in PSUM across row blocks; chunk j of C lives at
partition half (j < NCH/2 ? 0 : 64) so z^T fits in 4 banks and coexists
with the wave tiles.
"""

import math

import numpy as np

B, S_FULL, E, A, H = 2, 4096, 512, 8, 64
N_CORES = 8
NEG_BIG = -1.0e9

# Matmul dtype knobs:
#   ATTN_DT: scores (kt/qt) and z (vt/panel) matmuls — 'f32' | 'f32r' | 'bf16'
#   PROJ_DT: k/q/v + output projections            — 'f32' | 'f32r'
import os as _os

ATTN_DT = _os.environ.get("ATTN_DT", "bf16")
PROJ_DT = _os.environ.get("PROJ_DT", "f32r")
FILL_LDW = int(_os.environ.get("FILL_LDW", "0"))

_prog_cache = {}


def _build_program(S, attn_dt=None, proj_dt=None):
    import concourse.mybir as mybir
    import concourse.tile as tile
    from concourse import bacc

    attn_dt = attn_dt or ATTN_DT
    proj_dt = proj_dt or PROJ_DT
    f32 = mybir.dt.float32
    f32r = mybir.dt.float32r
    bf16 = mybir.dt.bfloat16
    # storage dtypes: engine-produced tensors carry the matmul dtype natively;
    # f32r DRAM tensors are DMA'd straight in (verifier accepts f32r DMA).
    # f32r matmuls cannot write PSUM at partition offset 64 (invalid ISA), so
    # the z stage (panel/vt), whose parity-1 chunks land there, drops to bf16
    # when attn_dt='f32r'; scores stay f32r.
    att_store = {"bf16": bf16, "f32r": f32r, "f32": f32}[attn_dt]
    z_store = bf16 if attn_dt == "bf16" or attn_dt == "f32r" else f32
    proj_store = {"f32r": f32r, "f32": f32}[proj_dt]

    EC = E // 128            # e chunks (contraction for projections)
    NCB = S // 128           # row blocks
    NCH = S // 512           # C chunks per full row
    HALF = NCH // 2          # chunks per partition half of z^T
    assert NCH % 2 == 0

    nc = bacc.Bacc("TRN2", target_bir_lowering=False, debug=False)

    xT = nc.dram_tensor("xT", [E, S], proj_store, kind="ExternalInput")
    wk = nc.dram_tensor("wk", [E, 128], proj_store, kind="ExternalInput")
    wq = nc.dram_tensor("wq", [E, 128], proj_store, kind="ExternalInput")
    wv = nc.dram_tensor("wv", [E, 128], proj_store, kind="ExternalInput")
    wo0 = nc.dram_tensor("wo0", [128, E], proj_store, kind="ExternalInput")
    wo1 = nc.dram_tensor("wo1", [128, E], proj_store, kind="ExternalInput")
    maskb = nc.dram_tensor("maskb", [128, 128], f32, kind="ExternalInput")
    outT = nc.dram_tensor("outT", [E, S], f32, kind="ExternalOutput")

    ExpF = mybir.ActivationFunctionType.Exp
    AxX = mybir.AxisListType.X
    AluAdd = mybir.AluOpType.add

    with tile.TileContext(nc) as tc:
        with (
            tc.tile_pool(name="singles", bufs=1) as singles,
            tc.tile_pool(name="panelp", bufs=2) as panelp,
            tc.tile_pool(name="zsbp", bufs=2) as zsbp,
            tc.tile_pool(name="small", bufs=6) as small,
            tc.tile_pool(name="outst", bufs=4) as outst,
            tc.tile_pool(name="ps", bufs=2, space="PSUM") as ps,
            tc.tile_pool(name="zps", bufs=1, space="PSUM") as zps,
        ):
            # ---- load inputs ----
            xt = singles.tile([128, EC, S], proj_store)
            wks = singles.tile([128, EC, 128], proj_store)
            wqs = singles.tile([128, EC, 128], proj_store)
            wvs = singles.tile([128, EC, 128], proj_store)
            for ec in range(EC):
                sl = slice(ec * 128, (ec + 1) * 128)
                nc.sync.dma_start(out=wks[:, ec, :], in_=wk[sl, :])
                nc.sync.dma_start(out=wqs[:, ec, :], in_=wq[sl, :])
                nc.sync.dma_start(out=wvs[:, ec, :], in_=wv[sl, :])
            SQ = S // 4
            for q in range(4):
                qsl = slice(q * SQ, (q + 1) * SQ)
                for ec in range(EC):
                    nc.sync.dma_start(
                        out=xt[:, ec, qsl], in_=xT[ec * 128:(ec + 1) * 128, qsl]
                    )
            wos0 = singles.tile([128, E], proj_store)
            wos1 = singles.tile([128, E], proj_store)
            nc.sync.dma_start(out=wos0, in_=wo0[:, :])
            nc.sync.dma_start(out=wos1, in_=wo1[:, :])
            msk = singles.tile([128, 128], f32)
            nc.sync.dma_start(out=msk, in_=maskb[:, :])
            # bf16 always: the K=1 zeroing matmuls are invalid ISA in f32r,
            # and mixing dtypes across an accumulation group is fine.
            zero_t = singles.tile([1, 576], bf16)
            nc.vector.memset(zero_t, 0.0)

            # ---- projections (quarter-major, following the x DMA stream) ----
            kt = singles.tile([128, S], att_store)
            qt = singles.tile([128, S], att_store)
            vsb = singles.tile([128, NCB, 128], f32)
            CPQ = NCH // 4           # 512-chunks per x quarter
            for q in range(4):
                for dst, w in ((kt, wks), (qt, wqs)):
                    for cc in range(q * CPQ, (q + 1) * CPQ):
                        csl = slice(cc * 512, (cc + 1) * 512)
                        wt = ps.tile([128, 1024], f32, tag="wave", name="wt")
                        for ec in range(EC):
                            nc.tensor.matmul(
                                wt[:, :512], w[:, ec, :], xt[:, ec, csl],
                                start=(ec == 0), stop=(ec == EC - 1),
                            )
                        nc.vector.tensor_copy(dst[:, csl], wt[:, :512])
                for cb in range(q * (NCB // 4), (q + 1) * (NCB // 4)):
                    csl = slice(cb * 128, (cb + 1) * 128)
                    wt = ps.tile([128, 1024], f32, tag="wave", name="wt")
                    for ec in range(EC):
                        nc.tensor.matmul(
                            wt[:, :128], xt[:, ec, csl], wvs[:, ec, :],
                            start=(ec == 0), stop=(ec == EC - 1),
                        )
                    nc.vector.tensor_copy(vsb[:, cb, :], wt[:, :128])

            # ---- attention per head ----
            # Software pipeline: z matmuls for row block cb are emitted two
            # blocks late, so PE streams scores while ACT/DVE finish the
            # exp + denominator chain. Head 1 sweeps row blocks in reverse so
            # its first (widest) score panels cover the head-0 z^T PSUM->SBUF
            # copy instead of stalling PE (which would re-throttle HAM).
            zsb_heads = []
            for h in range(2):
                hs = slice(h * 64, (h + 1) * 64)
                # wide/narrow interleave: every HAM window gets dense PE work,
                # and the leading wide block covers the head-boundary z copy
                order = []
                for i in range(NCB // 2):
                    order.append(NCB - 1 - i)
                    order.append(i)
                zT = zps.tile([128, HALF * 512], f32, name="zT")
                # Dummy matmuls zero the upper partition half of each z bank
                # (chunks j >= HALF accumulate with start=False onto these; a
                # real start=True there would clear the lower half's bank bits).
                for k in range(HALF):
                    nc.tensor.matmul(
                        zT[64:128, k * 512:(k + 1) * 512],
                        zero_t[:, :64], zero_t[:, 64:576],
                        start=True, stop=False, skip_group_check=True,
                    )

                first_cb = {}
                last_cb = {}
                for j in range(NCH):
                    part = [cb for cb in order if cb >= 4 * j]
                    first_cb[j] = part[0]
                    last_cb[j] = part[-1]

                def emit_z(item):
                    vt_i, panel_i, nch_i, cb_i = item
                    for j in range(nch_i):
                        poff = 0 if j < HALF else 64
                        col = (j % HALF) * 512
                        start = (j < HALF) and cb_i == first_cb[j]
                        stop = cb_i == last_cb[j]
                        nc.tensor.matmul(
                            zT[poff:poff + 64, col:col + 512],
                            vt_i,
                            panel_i[:, j * 512:(j + 1) * 512],
                            start=start, stop=stop,
                            skip_group_check=True,
                        )

                pending = []
                for cb in order:
                    c0 = cb * 128
                    nch = (c0 + 128 + 511) // 512
                    nwaves = (nch + 1) // 2
                    lastw = c0 + 128 - (nch - 1) * 512   # width of diag chunk
                    panel = panelp.tile([128, S], z_store, name="panel")
                    if lastw < 512:
                        # zero the diag chunk tail so z matmuls read zeros
                        nc.gpsimd.memset(
                            panel[:, (nch - 1) * 512 + lastw:nch * 512], 0.0
                        )
                    rsp = small.tile([128, 4], f32, name="rsp")
                    for wv_i in range(nwaves):
                        jlo = 2 * wv_i
                        jhi = min(jlo + 2, nch)
                        wt = ps.tile([128, 1024], f32, tag="wave", name="wt")
                        for j in range(jlo, jhi):
                            w_n = lastw if j == nch - 1 else 512
                            nc.tensor.matmul(
                                wt[:, (j - jlo) * 512:(j - jlo) * 512 + w_n],
                                kt[hs, c0:c0 + 128],
                                qt[hs, j * 512:j * 512 + w_n],
                                start=True, stop=True,
                            )
                        if jhi == nch:
                            # mask only the last 128 cols (the true triangle);
                            # earlier diag-chunk cols are fully valid
                            o = c0 - (nch - 1) * 512
                            dlo = (nch - 1 - jlo) * 512 + o
                            nc.vector.tensor_add(
                                wt[:, dlo:dlo + 128], wt[:, dlo:dlo + 128],
                                msk,
                            )
                        wlen = (jhi - jlo - 1) * 512 + (lastw if jhi == nch else 512)
                        nc.scalar.activation(
                            out=panel[:, jlo * 512:jlo * 512 + wlen],
                            in_=wt[:, :wlen],
                            func=ExpF,
                            scale=1.0 / math.sqrt(H),
                            accum_out=rsp[:, wv_i:wv_i + 1],
                        )
                    den = small.tile([128, 1], f32, name="den")
                    if nwaves > 1:
                        nc.vector.tensor_reduce(den, rsp[:, :nwaves], axis=AxX, op=AluAdd)
                    else:
                        nc.vector.tensor_copy(den, rsp[:, 0:1])
                    rden = small.tile([128, 1], f32, name="rden")
                    nc.vector.reciprocal(rden, den)
                    vt = small.tile([128, 64], z_store, name="vt")
                    nc.vector.tensor_scalar_mul(vt, vsb[:, cb, hs], rden)
                    pending.append((vt, panel, nch, cb))
                    if len(pending) > 2:
                        emit_z(pending.pop(0))
                    # dependency-free weight loads keep the PE activity monitor
                    # from re-throttling the clock during ACT-gated idles
                    for _ in range(FILL_LDW):
                        nc.tensor.ldweights(zero_t[:, :128])
                for item in pending:
                    emit_z(item)
                zsb = zsbp.tile([128, HALF * 512], proj_store, name="zsb")
                for q in range(HALF):
                    nc.vector.tensor_copy(
                        zsb[:, q * 512:(q + 1) * 512], zT[:, q * 512:(q + 1) * 512]
                    )
                zsb_heads.append(zsb)

            # ---- output projection: outT[e, C] = sum_ah WO[ah, e] z^T[ah, C]
            wos = (wos0, wos1)
            for ecn in range(EC):
                esl = slice(ecn * 128, (ecn + 1) * 128)
                for ccn in range(NCH):
                    poff = 0 if ccn < HALF else 64
                    col = (ccn % HALF) * 512
                    wt = ps.tile([128, 1024], f32, tag="wave", name="wt")
                    for h in range(2):
                        nc.tensor.matmul(
                            wt[:, :512],
                            wos[h][poff:poff + 64, esl],
                            zsb_heads[h][poff:poff + 64, col:col + 512],
                            start=(h == 0), stop=(h == 1),
                        )
                    st = outst.tile([128, 512], f32, name="st")
                    nc.vector.tensor_copy(st, wt[:, :512])
                    nc.sync.dma_start(
                        out=outT[esl, ccn * 512:(ccn + 1) * 512], in_=st
                    )

    nc.compile()
    return nc


def get_program(S=S_FULL):
    if S not in _prog_cache:
        _prog_cache[S] = _build_program(S)
    return _prog_cache[S]


def make_mask_band():
    """Triangle mask for the last 128 cols of a diagonal chunk:
    col t (relative to the diagonal start) is valid iff t <= r."""
    r = np.arange(128)[:, None]
    t = np.arange(128)[None, :]
    return np.where(t <= r, 0.0, NEG_BIG).astype(np.float32)


def make_core_inputs(x, W_K, W_Q, W_V, W_O, core):
    """Inputs for core = b*4 + g (batch b, head pair a0=2g, a1=2g+1)."""
    b, g = divmod(core, 4)
    a0, a1 = 2 * g, 2 * g + 1
    xT = np.ascontiguousarray(x[b].T)
    wk = np.ascontiguousarray(np.concatenate([W_K[a0].T, W_K[a1].T], axis=1))
    wq = np.ascontiguousarray(np.concatenate([W_Q[a0].T, W_Q[a1].T], axis=1))
    wv = np.ascontiguousarray(np.concatenate([W_V[a0].T, W_V[a1].T], axis=1))
    wo0 = np.ascontiguousarray(np.concatenate([W_O[a0].T, W_O[a0].T], axis=0))
    wo1 = np.ascontiguousarray(np.concatenate([W_O[a1].T, W_O[a1].T], axis=0))
    return {
        "xT": xT, "wk": wk, "wq": wq, "wv": wv,
        "wo0": wo0, "wo1": wo1, "maskb": make_mask_band(),
    }


def run_on_cores(inputs, trace=False):
    from concourse.bass_utils import run_bass_kernel_spmd

    nc = get_program()
    in_maps = [
        make_core_inputs(
            inputs["x"], inputs["W_K"], inputs["W_Q"], inputs["W_V"],
            inputs["W_O"], core,
        )
        for core in range(N_CORES)
    ]
    return run_bass_kernel_spmd(
        nc, in_maps, list(range(N_CORES)), trace=trace,
    )


def kernel(x, W_K, W_Q, W_V, W_O):
    x = np.asarray(x, dtype=np.float32)
    W_K = np.asarray(W_K, dtype=np.float32)
    W_Q = np.asarray(W_Q, dtype=np.float32)
    W_V = np.asarray(W_V, dtype=np.float32)
    W_O = np.asarray(W_O, dtype=np.float32)
    res = run_on_cores(
        {"x": x, "W_K": W_K, "W_Q": W_Q, "W_V": W_V, "W_O": W_O}
    )
    out = np.zeros((B, S_FULL, E), dtype=np.float32)
    for b in range(B):
        acc = np.zeros((E, S_FULL), dtype=np.float32)
        for g in range(4):
            acc += res.results[b * 4 + g]["outT"]
        out[b] = acc.T
    return out


# revision 17
# speedup vs baseline: 1.1690x; 1.1690x over previous
"""Trainium2 Bass kernel for the quirky multi-head attention problem.

Math (per batch b, head a), faithful to the reference:
    K = x[b] @ W_K[a].T          # [S, H]
    Q = x[b] @ W_Q[a].T
    V = x[b] @ W_V[a].T
    s[c, C] = (K @ Q.T)[c, C] / sqrt(H)        rows c = "key" index
    valid iff C <= c (tril); softmax over C per row c
    E = exp(s) * tril            # no max-subtraction: |s| <= ~7, fp32-safe
    denom[c] = sum_C E[c, C]
    z[C, h] = sum_c E[c, C] * (V/denom)[c, h]  # = E.T @ (V/denom)
    out[b] += z @ W_O[a].T

Sharding: 8 cores = 2 batches x 4 head-pairs. Each core handles one batch
and two heads; the attention matrix is device-local. Host sums the four
head-pair partial outputs per batch.

Device layouts (per core):
    xT   [E=512, S]   x[b] transposed (host-side)
    wk/wq/wv [E, 128] W[a0].T | W[a1].T concat on head axis
    wo0/wo1 [128, E]  W_O[a].T duplicated on both partition halves
    maskb [128, 896]  additive causal mask bands (0 or -1e9)
    outT [E, S]       partial output, transposed

On-chip flow per head: scores [c_blk=128 rows, 512-wide C chunks] are
matmul'd into rotating PSUM wave tiles (2 banks x 2 bufs), the diagonal
chunk gets an additive -1e9 mask, ScalarE applies exp (scale=1/8) writing
the row panel to SBUF with a fused per-row accumulation (softmax denom).
z^T [64, S] accumulates in PSUM across row blocks; chunk j of C lives at
partition half (j < NCH/2 ? 0 : 64) so z^T fits in 4 banks and coexists
with the wave tiles.
"""

import math

import numpy as np

B, S_FULL, E, A, H = 2, 4096, 512, 8, 64
N_CORES = 8
NEG_BIG = -1.0e9

# Matmul dtype knobs:
#   ATTN_DT: scores (kt/qt) and z (vt/panel) matmuls — 'f32' | 'f32r' | 'bf16'
#   PROJ_DT: k/q/v + output projections            — 'f32' | 'f32r'
import os as _os

ATTN_DT = _os.environ.get("ATTN_DT", "bf16")
PROJ_DT = _os.environ.get("PROJ_DT", "f32r")
FILL_LDW = int(_os.environ.get("FILL_LDW", "0"))
Z_LAG = int(_os.environ.get("Z_LAG", "3"))
PANEL_BUFS = int(_os.environ.get("PANEL_BUFS", "4"))

_prog_cache = {}


def _build_program(S, attn_dt=None, proj_dt=None):
    import concourse.mybir as mybir
    import concourse.tile as tile
    from concourse import bacc

    attn_dt = attn_dt or ATTN_DT
    proj_dt = proj_dt or PROJ_DT
    f32 = mybir.dt.float32
    f32r = mybir.dt.float32r
    bf16 = mybir.dt.bfloat16
    # storage dtypes: engine-produced tensors carry the matmul dtype natively;
    # f32r DRAM tensors are DMA'd straight in (verifier accepts f32r DMA).
    # f32r matmuls cannot write PSUM at partition offset 64 (invalid ISA), so
    # the z stage (panel/vt), whose parity-1 chunks land there, drops to bf16
    # when attn_dt='f32r'; scores stay f32r.
    att_store = {"bf16": bf16, "f32r": f32r, "f32": f32}[attn_dt]
    z_store = bf16 if attn_dt == "bf16" or attn_dt == "f32r" else f32
    proj_store = {"f32r": f32r, "f32": f32}[proj_dt]

    EC = E // 128            # e chunks (contraction for projections)
    NCB = S // 128           # row blocks
    NCH = S // 512           # C chunks per full row
    HALF = NCH // 2          # chunks per partition half of z^T
    assert NCH % 2 == 0

    nc = bacc.Bacc("TRN2", target_bir_lowering=False, debug=False)

    xT = nc.dram_tensor("xT", [E, S], proj_store, kind="ExternalInput")
    wk = nc.dram_tensor("wk", [E, 128], proj_store, kind="ExternalInput")
    wq = nc.dram_tensor("wq", [E, 128], proj_store, kind="ExternalInput")
    wv = nc.dram_tensor("wv", [E, 128], proj_store, kind="ExternalInput")
    wo0 = nc.dram_tensor("wo0", [128, E], proj_store, kind="ExternalInput")
    wo1 = nc.dram_tensor("wo1", [128, E], proj_store, kind="ExternalInput")
    maskb = nc.dram_tensor("maskb", [128, 128], f32, kind="ExternalInput")
    outT = nc.dram_tensor("outT", [E, S], f32, kind="ExternalOutput")

    ExpF = mybir.ActivationFunctionType.Exp
    AxX = mybir.AxisListType.X
    AluAdd = mybir.AluOpType.add

    with tile.TileContext(nc) as tc:
        with (
            tc.tile_pool(name="singles", bufs=1) as singles,
            tc.tile_pool(name="panelp", bufs=PANEL_BUFS) as panelp,
            tc.tile_pool(name="zsbp", bufs=2) as zsbp,
            tc.tile_pool(name="small", bufs=8) as small,
            tc.tile_pool(name="outst", bufs=4) as outst,
            tc.tile_pool(name="ps", bufs=2, space="PSUM") as ps,
            tc.tile_pool(name="zps", bufs=1, space="PSUM") as zps,
        ):
            # ---- load inputs ----
            xt = singles.tile([128, EC, S], proj_store)
            wks = singles.tile([128, EC, 128], proj_store)
            wqs = singles.tile([128, EC, 128], proj_store)
            wvs = singles.tile([128, EC, 128], proj_store)
            for ec in range(EC):
                sl = slice(ec * 128, (ec + 1) * 128)
                nc.sync.dma_start(out=wks[:, ec, :], in_=wk[sl, :])
                nc.sync.dma_start(out=wqs[:, ec, :], in_=wq[sl, :])
                nc.sync.dma_start(out=wvs[:, ec, :], in_=wv[sl, :])
            SQ = S // 4
            for q in range(4):
                qsl = slice(q * SQ, (q + 1) * SQ)
                for ec in range(EC):
                    nc.sync.dma_start(
                        out=xt[:, ec, qsl], in_=xT[ec * 128:(ec + 1) * 128, qsl]
                    )
            wos0 = singles.tile([128, E], proj_store)
            wos1 = singles.tile([128, E], proj_store)
            nc.sync.dma_start(out=wos0, in_=wo0[:, :])
            nc.sync.dma_start(out=wos1, in_=wo1[:, :])
            msk = singles.tile([128, 128], f32)
            nc.sync.dma_start(out=msk, in_=maskb[:, :])
            # bf16 always: the K=1 zeroing matmuls are invalid ISA in f32r,
            # and mixing dtypes across an accumulation group is fine.
            zero_t = singles.tile([1, 576], bf16)
            nc.vector.memset(zero_t, 0.0)

            # ---- projections (quarter-major, following the x DMA stream) ----
            kt = singles.tile([128, S], att_store)
            qt = singles.tile([128, S], att_store)
            vsb = singles.tile([128, NCB, 128], f32)
            CPQ = NCH // 4           # 512-chunks per x quarter
            for q in range(4):
                for dst, w in ((kt, wks), (qt, wqs)):
                    for cc in range(q * CPQ, (q + 1) * CPQ):
                        csl = slice(cc * 512, (cc + 1) * 512)
                        wt = ps.tile([128, 1024], f32, tag="wave", name="wt")
                        for ec in range(EC):
                            nc.tensor.matmul(
                                wt[:, :512], w[:, ec, :], xt[:, ec, csl],
                                start=(ec == 0), stop=(ec == EC - 1),
                            )
                        nc.vector.tensor_copy(dst[:, csl], wt[:, :512])
                for cb in range(q * (NCB // 4), (q + 1) * (NCB // 4)):
                    csl = slice(cb * 128, (cb + 1) * 128)
                    wt = ps.tile([128, 1024], f32, tag="wave", name="wt")
                    for ec in range(EC):
                        nc.tensor.matmul(
                            wt[:, :128], xt[:, ec, csl], wvs[:, ec, :],
                            start=(ec == 0), stop=(ec == EC - 1),
                        )
                    nc.vector.tensor_copy(vsb[:, cb, :], wt[:, :128])

            # ---- attention per head ----
            # Software pipeline: z matmuls for row block cb are emitted two
            # blocks late, so PE streams scores while ACT/DVE finish the
            # exp + denominator chain. Head 1 sweeps row blocks in reverse so
            # its first (widest) score panels cover the head-0 z^T PSUM->SBUF
            # copy instead of stalling PE (which would re-throttle HAM).
            zsb_heads = []
            for h in range(2):
                hs = slice(h * 64, (h + 1) * 64)
                # head 0 forward, head 1 reverse: the reverse head leads with
                # its widest blocks, covering the head-boundary z^T copy
                order = list(range(NCB - 1, -1, -1)) if h == 1 else list(range(NCB))
                zT = zps.tile([128, HALF * 512], f32, name="zT")
                # Dummy matmuls zero the upper partition half of each z bank
                # (chunks j >= HALF accumulate with start=False onto these; a
                # real start=True there would clear the lower half's bank bits).
                for k in range(HALF):
                    nc.tensor.matmul(
                        zT[64:128, k * 512:(k + 1) * 512],
                        zero_t[:, :64], zero_t[:, 64:576],
                        start=True, stop=False, skip_group_check=True,
                    )

                first_cb = {}
                last_cb = {}
                for j in range(NCH):
                    part = [cb for cb in order if cb >= 4 * j]
                    first_cb[j] = part[0]
                    last_cb[j] = part[-1]

                def emit_z(item):
                    vt_i, panel_i, nch_i, cb_i = item
                    for j in range(nch_i):
                        poff = 0 if j < HALF else 64
                        col = (j % HALF) * 512
                        start = (j < HALF) and cb_i == first_cb[j]
                        stop = cb_i == last_cb[j]
                        nc.tensor.matmul(
                            zT[poff:poff + 64, col:col + 512],
                            vt_i,
                            panel_i[:, j * 512:(j + 1) * 512],
                            start=start, stop=stop,
                            skip_group_check=True,
                        )

                pending = []
                for cb in order:
                    c0 = cb * 128
                    nch = (c0 + 128 + 511) // 512
                    nwaves = (nch + 1) // 2
                    lastw = c0 + 128 - (nch - 1) * 512   # width of diag chunk
                    panel = panelp.tile([128, S], z_store, name="panel")
                    if lastw < 512:
                        # zero the diag chunk tail so z matmuls read zeros
                        nc.gpsimd.memset(
                            panel[:, (nch - 1) * 512 + lastw:nch * 512], 0.0
                        )
                    rsp = small.tile([128, 4], f32, name="rsp")
                    for wv_i in range(nwaves):
                        jlo = 2 * wv_i
                        jhi = min(jlo + 2, nch)
                        wt = ps.tile([128, 1024], f32, tag="wave", name="wt")
                        for j in range(jlo, jhi):
                            w_n = lastw if j == nch - 1 else 512
                            nc.tensor.matmul(
                                wt[:, (j - jlo) * 512:(j - jlo) * 512 + w_n],
                                kt[hs, c0:c0 + 128],
                                qt[hs, j * 512:j * 512 + w_n],
                                start=True, stop=True,
                            )
                        if jhi == nch:
                            # mask only the last 128 cols (the true triangle);
                            # earlier diag-chunk cols are fully valid
                            o = c0 - (nch - 1) * 512
                            dlo = (nch - 1 - jlo) * 512 + o
                            nc.vector.tensor_add(
                                wt[:, dlo:dlo + 128], wt[:, dlo:dlo + 128],
                                msk,
                            )
                        wlen = (jhi - jlo - 1) * 512 + (lastw if jhi == nch else 512)
                        nc.scalar.activation(
                            out=panel[:, jlo * 512:jlo * 512 + wlen],
                            in_=wt[:, :wlen],
                            func=ExpF,
                            scale=1.0 / math.sqrt(H),
                            accum_out=rsp[:, wv_i:wv_i + 1],
                        )
                    den = small.tile([128, 1], f32, name="den")
                    if nwaves > 1:
                        nc.vector.tensor_reduce(den, rsp[:, :nwaves], axis=AxX, op=AluAdd)
                    else:
                        nc.vector.tensor_copy(den, rsp[:, 0:1])
                    rden = small.tile([128, 1], f32, name="rden")
                    nc.vector.reciprocal(rden, den)
                    vt = small.tile([128, 64], z_store, name="vt")
                    nc.vector.tensor_scalar_mul(vt, vsb[:, cb, hs], rden)
                    pending.append((vt, panel, nch, cb))
                    if len(pending) > Z_LAG:
                        emit_z(pending.pop(0))
                    # dependency-free weight loads keep the PE activity monitor
                    # from re-throttling the clock during ACT-gated idles
                    for _ in range(FILL_LDW):
                        nc.tensor.ldweights(zero_t[:, :128])
                for item in pending:
                    emit_z(item)
                zsb = zsbp.tile([128, HALF * 512], proj_store, name="zsb")
                for q in range(HALF):
                    nc.vector.tensor_copy(
                        zsb[:, q * 512:(q + 1) * 512], zT[:, q * 512:(q + 1) * 512]
                    )
                zsb_heads.append(zsb)

            # ---- output projection: outT[e, C] = sum_ah WO[ah, e] z^T[ah, C]
            wos = (wos0, wos1)
            for ecn in range(EC):
                esl = slice(ecn * 128, (ecn + 1) * 128)
                for ccn in range(NCH):
                    poff = 0 if ccn < HALF else 64
                    col = (ccn % HALF) * 512
                    wt = ps.tile([128, 1024], f32, tag="wave", name="wt")
                    for h in range(2):
                        nc.tensor.matmul(
                            wt[:, :512],
                            wos[h][poff:poff + 64, esl],
                            zsb_heads[h][poff:poff + 64, col:col + 512],
                            start=(h == 0), stop=(h == 1),
                        )
                    st = outst.tile([128, 512], f32, name="st")
                    nc.vector.tensor_copy(st, wt[:, :512])
                    nc.sync.dma_start(
                        out=outT[esl, ccn * 512:(ccn + 1) * 512], in_=st
                    )

    nc.compile()
    return nc


def get_program(S=S_FULL):
    if S not in _prog_cache:
        _prog_cache[S] = _build_program(S)
    return _prog_cache[S]


def make_mask_band():
    """Triangle mask for the last 128 cols of a diagonal chunk:
    col t (relative to the diagonal start) is valid iff t <= r."""
    r = np.arange(128)[:, None]
    t = np.arange(128)[None, :]
    return np.where(t <= r, 0.0, NEG_BIG).astype(np.float32)


def make_core_inputs(x, W_K, W_Q, W_V, W_O, core):
    """Inputs for core = b*4 + g (batch b, head pair a0=2g, a1=2g+1)."""
    b, g = divmod(core, 4)
    a0, a1 = 2 * g, 2 * g + 1
    xT = np.ascontiguousarray(x[b].T)
    wk = np.ascontiguousarray(np.concatenate([W_K[a0].T, W_K[a1].T], axis=1))
    wq = np.ascontiguousarray(np.concatenate([W_Q[a0].T, W_Q[a1].T], axis=1))
    wv = np.ascontiguousarray(np.concatenate([W_V[a0].T, W_V[a1].T], axis=1))
    wo0 = np.ascontiguousarray(np.concatenate([W_O[a0].T, W_O[a0].T], axis=0))
    wo1 = np.ascontiguousarray(np.concatenate([W_O[a1].T, W_O[a1].T], axis=0))
    return {
        "xT": xT, "wk": wk, "wq": wq, "wv": wv,
        "wo0": wo0, "wo1": wo1, "maskb": make_mask_band(),
    }


def run_on_cores(inputs, trace=False):
    from concourse.bass_utils import run_bass_kernel_spmd

    nc = get_program()
    in_maps = [
        make_core_inputs(
            inputs["x"], inputs["W_K"], inputs["W_Q"], inputs["W_V"],
            inputs["W_O"], core,
        )
        for core in range(N_CORES)
    ]
    return run_bass_kernel_spmd(
        nc, in_maps, list(range(N_CORES)), trace=trace,
    )


def kernel(x, W_K, W_Q, W_V, W_O):
    x = np.asarray(x, dtype=np.float32)
    W_K = np.asarray(W_K, dtype=np.float32)
    W_Q = np.asarray(W_Q, dtype=np.float32)
    W_V = np.asarray(W_V, dtype=np.float32)
    W_O = np.asarray(W_O, dtype=np.float32)
    res = run_on_cores(
        {"x": x, "W_K": W_K, "W_Q": W_Q, "W_V": W_V, "W_O": W_O}
    )
    out = np.zeros((B, S_FULL, E), dtype=np.float32)
    for b in range(B):
        acc = np.zeros((E, S_FULL), dtype=np.float32)
        for g in range(4):
            acc += res.results[b * 4 + g]["outT"]
        out[b] = acc.T
    return out


# revision 19
# speedup vs baseline: 1.1961x; 1.0232x over previous
"""Trainium2 Bass kernel for the quirky multi-head attention problem.

Math (per batch b, head a), faithful to the reference:
    K = x[b] @ W_K[a].T          # [S, H]
    Q = x[b] @ W_Q[a].T
    V = x[b] @ W_V[a].T
    s[c, C] = (K @ Q.T)[c, C] / sqrt(H)        rows c = "key" index
    valid iff C <= c (tril); softmax over C per row c
    E = exp(s) * tril            # no max-subtraction: |s| <= ~7, fp32-safe
    denom[c] = sum_C E[c, C]
    z[C, h] = sum_c E[c, C] * (V/denom)[c, h]  # = E.T @ (V/denom)
    out[b] += z @ W_O[a].T

Sharding: 8 cores = 2 batches x 4 head-pairs. Each core handles one batch
and two heads; the attention matrix is device-local. Host sums the four
head-pair partial outputs per batch.

Device layouts (per core):
    xT   [E=512, S]   x[b] transposed (host-side)
    wk/wq/wv [E, 128] W[a0].T | W[a1].T concat on head axis
    wo0/wo1 [128, E]  W_O[a].T duplicated on both partition halves
    maskb [128, 896]  additive causal mask bands (0 or -1e9)
    outT [E, S]       partial output, transposed

On-chip flow per head: scores [c_blk=128 rows, 512-wide C chunks] are
matmul'd into rotating PSUM wave tiles (2 banks x 2 bufs), the diagonal
chunk gets an additive -1e9 mask, ScalarE applies exp (scale=1/8) writing
the row panel to SBUF with a fused per-row accumulation (softmax denom).
z^T [64, S] accumulates in PSUM across row blocks; chunk j of C lives at
partition half (j < NCH/2 ? 0 : 64) so z^T fits in 4 banks and coexists
with the wave tiles.
"""

import math

import numpy as np

B, S_FULL, E, A, H = 2, 4096, 512, 8, 64
N_CORES = 8
NEG_BIG = -1.0e9

# Matmul dtype knobs:
#   ATTN_DT: scores (kt/qt) and z (vt/panel) matmuls — 'f32' | 'f32r' | 'bf16'
#   PROJ_DT: k/q/v + output projections            — 'f32' | 'f32r'
import os as _os

ATTN_DT = _os.environ.get("ATTN_DT", "bf16")
PROJ_DT = _os.environ.get("PROJ_DT", "f32r")
FILL_LDW = int(_os.environ.get("FILL_LDW", "0"))
Z_LAG = int(_os.environ.get("Z_LAG", "3"))
PANEL_BUFS = int(_os.environ.get("PANEL_BUFS", "4"))

_prog_cache = {}


def _build_program(S, attn_dt=None, proj_dt=None):
    import concourse.mybir as mybir
    import concourse.tile as tile
    from concourse import bacc

    attn_dt = attn_dt or ATTN_DT
    proj_dt = proj_dt or PROJ_DT
    f32 = mybir.dt.float32
    f32r = mybir.dt.float32r
    bf16 = mybir.dt.bfloat16
    # storage dtypes: engine-produced tensors carry the matmul dtype natively;
    # f32r DRAM tensors are DMA'd straight in (verifier accepts f32r DMA).
    # f32r matmuls cannot write PSUM at partition offset 64 (invalid ISA), so
    # the z stage (panel/vt), whose parity-1 chunks land there, drops to bf16
    # when attn_dt='f32r'; scores stay f32r.
    att_store = {"bf16": bf16, "f32r": f32r, "f32": f32}[attn_dt]
    z_store = bf16 if attn_dt == "bf16" or attn_dt == "f32r" else f32
    proj_store = {"f32r": f32r, "f32": f32}[proj_dt]

    EC = E // 128            # e chunks (contraction for projections)
    NCB = S // 128           # row blocks
    NCH = S // 512           # C chunks per full row
    HALF = NCH // 2          # chunks per partition half of z^T
    assert NCH % 2 == 0

    nc = bacc.Bacc("TRN2", target_bir_lowering=False, debug=False)

    xT = nc.dram_tensor("xT", [E, S], proj_store, kind="ExternalInput")
    wk = nc.dram_tensor("wk", [E, 128], proj_store, kind="ExternalInput")
    wq = nc.dram_tensor("wq", [E, 128], proj_store, kind="ExternalInput")
    wv = nc.dram_tensor("wv", [E, 128], proj_store, kind="ExternalInput")
    wo0 = nc.dram_tensor("wo0", [128, E], proj_store, kind="ExternalInput")
    wo1 = nc.dram_tensor("wo1", [128, E], proj_store, kind="ExternalInput")
    maskb = nc.dram_tensor("maskb", [128, 128], f32, kind="ExternalInput")
    outT = nc.dram_tensor("outT", [E, S], f32, kind="ExternalOutput")

    ExpF = mybir.ActivationFunctionType.Exp
    AxX = mybir.AxisListType.X
    AluAdd = mybir.AluOpType.add

    with tile.TileContext(nc) as tc:
        with (
            tc.tile_pool(name="singles", bufs=1) as singles,
            tc.tile_pool(name="panelp", bufs=PANEL_BUFS) as panelp,
            tc.tile_pool(name="zsbp", bufs=2) as zsbp,
            tc.tile_pool(name="small", bufs=8) as small,
            tc.tile_pool(name="outst", bufs=4) as outst,
            tc.tile_pool(name="ps", bufs=2, space="PSUM") as ps,
            tc.tile_pool(name="zps", bufs=1, space="PSUM") as zps,
        ):
            # ---- load inputs ----
            xt = singles.tile([128, EC, S], proj_store)
            wks = singles.tile([128, EC, 128], proj_store)
            wqs = singles.tile([128, EC, 128], proj_store)
            wvs = singles.tile([128, EC, 128], proj_store)
            for ec in range(EC):
                sl = slice(ec * 128, (ec + 1) * 128)
                nc.sync.dma_start(out=wks[:, ec, :], in_=wk[sl, :])
                nc.sync.dma_start(out=wqs[:, ec, :], in_=wq[sl, :])
                nc.sync.dma_start(out=wvs[:, ec, :], in_=wv[sl, :])
            SQ = S // 4
            for q in range(4):
                qsl = slice(q * SQ, (q + 1) * SQ)
                for ec in range(EC):
                    nc.sync.dma_start(
                        out=xt[:, ec, qsl], in_=xT[ec * 128:(ec + 1) * 128, qsl]
                    )
            wos0 = singles.tile([128, E], proj_store)
            wos1 = singles.tile([128, E], proj_store)
            nc.sync.dma_start(out=wos0, in_=wo0[:, :])
            nc.sync.dma_start(out=wos1, in_=wo1[:, :])
            msk = singles.tile([128, 128], f32)
            nc.sync.dma_start(out=msk, in_=maskb[:, :])
            # bf16 always: the K=1 zeroing matmuls are invalid ISA in f32r,
            # and mixing dtypes across an accumulation group is fine.
            zero_t = singles.tile([1, 576], bf16)
            nc.vector.memset(zero_t, 0.0)

            # ---- projections (quarter-major, following the x DMA stream) ----
            kt = singles.tile([128, S], att_store)
            qt = singles.tile([128, S], att_store)
            vsb = singles.tile([128, NCB, 128], f32)
            CPQ = NCH // 4           # 512-chunks per x quarter
            for q in range(4):
                for dst, w in ((kt, wks), (qt, wqs)):
                    for cc in range(q * CPQ, (q + 1) * CPQ):
                        csl = slice(cc * 512, (cc + 1) * 512)
                        wt = ps.tile([128, 1024], f32, tag="wave", name="wt")
                        for ec in range(EC):
                            nc.tensor.matmul(
                                wt[:, :512], w[:, ec, :], xt[:, ec, csl],
                                start=(ec == 0), stop=(ec == EC - 1),
                            )
                        nc.vector.tensor_copy(dst[:, csl], wt[:, :512])
                for cb in range(q * (NCB // 4), (q + 1) * (NCB // 4)):
                    csl = slice(cb * 128, (cb + 1) * 128)
                    wt = ps.tile([128, 1024], f32, tag="wave", name="wt")
                    for ec in range(EC):
                        nc.tensor.matmul(
                            wt[:, :128], xt[:, ec, csl], wvs[:, ec, :],
                            start=(ec == 0), stop=(ec == EC - 1),
                        )
                    nc.vector.tensor_copy(vsb[:, cb, :], wt[:, :128])

            wos = (wos0, wos1)

            # ---- attention per head ----
            # Software pipeline: z matmuls for row block cb are emitted two
            # blocks late, so PE streams scores while ACT/DVE finish the
            # exp + denominator chain. Head 1 sweeps row blocks in reverse so
            # its first (widest) score panels cover the head-0 z^T PSUM->SBUF
            # copy instead of stalling PE (which would re-throttle HAM).
            zsb_heads = []
            for h in range(2):
                hs = slice(h * 64, (h + 1) * 64)
                # head 0 forward, head 1 reverse: the reverse head leads with
                # its widest blocks, covering the head-boundary z^T copy
                order = list(range(NCB - 1, -1, -1)) if h == 1 else list(range(NCB))
                zT = zps.tile([128, HALF * 512], f32, name="zT")
                # Dummy matmuls zero the upper partition half of each z bank
                # (chunks j >= HALF accumulate with start=False onto these; a
                # real start=True there would clear the lower half's bank bits).
                for k in range(HALF):
                    nc.tensor.matmul(
                        zT[64:128, k * 512:(k + 1) * 512],
                        zero_t[:, :64], zero_t[:, 64:576],
                        start=True, stop=False, skip_group_check=True,
                    )

                first_cb = {}
                last_cb = {}
                for j in range(NCH):
                    part = [cb for cb in order if cb >= 4 * j]
                    first_cb[j] = part[0]
                    last_cb[j] = part[-1]

                def emit_z(item):
                    vt_i, panel_i, nch_i, cb_i = item
                    for j in range(nch_i):
                        poff = 0 if j < HALF else 64
                        col = (j % HALF) * 512
                        start = (j < HALF) and cb_i == first_cb[j]
                        stop = cb_i == last_cb[j]
                        nc.tensor.matmul(
                            zT[poff:poff + 64, col:col + 512],
                            vt_i,
                            panel_i[:, j * 512:(j + 1) * 512],
                            start=start, stop=stop,
                            skip_group_check=True,
                        )
                    # Head 1 runs in reverse, so once cb=4q is done, z bank q
                    # (chunks q and q+HALF, both heads) is final: trickle the
                    # z copy + output projection into the narrow-block tail,
                    # where PE would otherwise starve.
                    if h == 1 and cb_i % 4 == 0 and cb_i // 4 < HALF:
                        q = cb_i // 4
                        nc.vector.tensor_copy(
                            zsb[:, q * 512:(q + 1) * 512],
                            zT[:, q * 512:(q + 1) * 512],
                        )
                        emit_out_proj(q)

                zsb = zsbp.tile([128, HALF * 512], proj_store, name="zsb")
                zsb_heads.append(zsb)

                def emit_out_proj(q):
                    for ccn in (q, q + HALF):
                        poff = 0 if ccn < HALF else 64
                        col = q * 512
                        for ecn in range(EC):
                            esl = slice(ecn * 128, (ecn + 1) * 128)
                            wt = ps.tile([128, 1024], f32, tag="wave", name="wt")
                            for hh in range(2):
                                nc.tensor.matmul(
                                    wt[:, :512],
                                    wos[hh][poff:poff + 64, esl],
                                    zsb_heads[hh][poff:poff + 64, col:col + 512],
                                    start=(hh == 0), stop=(hh == 1),
                                )
                            st = outst.tile([128, 512], f32, name="st")
                            nc.vector.tensor_copy(st, wt[:, :512])
                            nc.sync.dma_start(
                                out=outT[esl, ccn * 512:(ccn + 1) * 512], in_=st
                            )

                pending = []
                for cb in order:
                    c0 = cb * 128
                    nch = (c0 + 128 + 511) // 512
                    nwaves = (nch + 1) // 2
                    lastw = c0 + 128 - (nch - 1) * 512   # width of diag chunk
                    panel = panelp.tile([128, S], z_store, name="panel")
                    if lastw < 512:
                        # zero the diag chunk tail so z matmuls read zeros
                        nc.gpsimd.memset(
                            panel[:, (nch - 1) * 512 + lastw:nch * 512], 0.0
                        )
                    rsp = small.tile([128, 4], f32, name="rsp")
                    for wv_i in range(nwaves):
                        jlo = 2 * wv_i
                        jhi = min(jlo + 2, nch)
                        wt = ps.tile([128, 1024], f32, tag="wave", name="wt")
                        for j in range(jlo, jhi):
                            w_n = lastw if j == nch - 1 else 512
                            nc.tensor.matmul(
                                wt[:, (j - jlo) * 512:(j - jlo) * 512 + w_n],
                                kt[hs, c0:c0 + 128],
                                qt[hs, j * 512:j * 512 + w_n],
                                start=True, stop=True,
                            )
                        if jhi == nch:
                            # mask only the last 128 cols (the true triangle);
                            # earlier diag-chunk cols are fully valid
                            o = c0 - (nch - 1) * 512
                            dlo = (nch - 1 - jlo) * 512 + o
                            nc.vector.tensor_add(
                                wt[:, dlo:dlo + 128], wt[:, dlo:dlo + 128],
                                msk,
                            )
                        wlen = (jhi - jlo - 1) * 512 + (lastw if jhi == nch else 512)
                        nc.scalar.activation(
                            out=panel[:, jlo * 512:jlo * 512 + wlen],
                            in_=wt[:, :wlen],
                            func=ExpF,
                            scale=1.0 / math.sqrt(H),
                            accum_out=rsp[:, wv_i:wv_i + 1],
                        )
                    den = small.tile([128, 1], f32, name="den")
                    if nwaves > 1:
                        nc.vector.tensor_reduce(den, rsp[:, :nwaves], axis=AxX, op=AluAdd)
                    else:
                        nc.vector.tensor_copy(den, rsp[:, 0:1])
                    rden = small.tile([128, 1], f32, name="rden")
                    nc.vector.reciprocal(rden, den)
                    vt = small.tile([128, 64], z_store, name="vt")
                    nc.vector.tensor_scalar_mul(vt, vsb[:, cb, hs], rden)
                    pending.append((vt, panel, nch, cb))
                    if len(pending) > Z_LAG:
                        emit_z(pending.pop(0))
                    # dependency-free weight loads keep the PE activity monitor
                    # from re-throttling the clock during ACT-gated idles
                    for _ in range(FILL_LDW):
                        nc.tensor.ldweights(zero_t[:, :128])
                for item in pending:
                    emit_z(item)
                if h == 0:
                    for q in range(HALF):
                        nc.vector.tensor_copy(
                            zsb[:, q * 512:(q + 1) * 512],
                            zT[:, q * 512:(q + 1) * 512],
                        )

    nc.compile()
    return nc


def get_program(S=S_FULL):
    if S not in _prog_cache:
        _prog_cache[S] = _build_program(S)
    return _prog_cache[S]


def make_mask_band():
    """Triangle mask for the last 128 cols of a diagonal chunk:
    col t (relative to the diagonal start) is valid iff t <= r."""
    r = np.arange(128)[:, None]
    t = np.arange(128)[None, :]
    return np.where(t <= r, 0.0, NEG_BIG).astype(np.float32)


def make_core_inputs(x, W_K, W_Q, W_V, W_O, core):
    """Inputs for core = b*4 + g (batch b, head pair a0=2g, a1=2g+1)."""
    b, g = divmod(core, 4)
    a0, a1 = 2 * g, 2 * g + 1
    xT = np.ascontiguousarray(x[b].T)
    wk = np.ascontiguousarray(np.concatenate([W_K[a0].T, W_K[a1].T], axis=1))
    wq = np.ascontiguousarray(np.concatenate([W_Q[a0].T, W_Q[a1].T], axis=1))
    wv = np.ascontiguousarray(np.concatenate([W_V[a0].T, W_V[a1].T], axis=1))
    wo0 = np.ascontiguousarray(np.concatenate([W_O[a0].T, W_O[a0].T], axis=0))
    wo1 = np.ascontiguousarray(np.concatenate([W_O[a1].T, W_O[a1].T], axis=0))
    return {
        "xT": xT, "wk": wk, "wq": wq, "wv": wv,
        "wo0": wo0, "wo1": wo1, "maskb": make_mask_band(),
    }


def run_on_cores(inputs, trace=False):
    from concourse.bass_utils import run_bass_kernel_spmd

    nc = get_program()
    in_maps = [
        make_core_inputs(
            inputs["x"], inputs["W_K"], inputs["W_Q"], inputs["W_V"],
            inputs["W_O"], core,
        )
        for core in range(N_CORES)
    ]
    return run_bass_kernel_spmd(
        nc, in_maps, list(range(N_CORES)), trace=trace,
    )


def kernel(x, W_K, W_Q, W_V, W_O):
    x = np.asarray(x, dtype=np.float32)
    W_K = np.asarray(W_K, dtype=np.float32)
    W_Q = np.asarray(W_Q, dtype=np.float32)
    W_V = np.asarray(W_V, dtype=np.float32)
    W_O = np.asarray(W_O, dtype=np.float32)
    res = run_on_cores(
        {"x": x, "W_K": W_K, "W_Q": W_Q, "W_V": W_V, "W_O": W_O}
    )
    out = np.zeros((B, S_FULL, E), dtype=np.float32)
    for b in range(B):
        acc = np.zeros((E, S_FULL), dtype=np.float32)
        for g in range(4):
            acc += res.results[b * 4 + g]["outT"]
        out[b] = acc.T
    return out
